# revision 39
# baseline (speedup 1.0000x reference)
"""Trainium2 Bass kernel for ChunkLevelFeatureEncoderNERCNN.

The reference gathers ragged chunks (len 0..4, truncated to K=8) from a
contiguous token stream, runs conv1d(k=3, pad=1) + bias + relu per chunk,
and scatters results back.  Chunk start offsets advance by the full chunk
length, so the gathered tokens are a known (host-computable) subsequence of
the input rows and the op is a *chunk-blocked* conv1d over that sequence:

    y[p] = relu(b + W1 @ x[p] + ml[p] * W0 @ x[p-1] + mr[p] * W2 @ x[p+1])

where ml/mr zero the taps that cross a chunk boundary.  Because the masks
handle arbitrary boundaries, all covered tokens of all 32 batches are
concatenated into ONE global stream, split at chunk boundaries into 8
near-equal spans (one per NeuronCore) — no per-batch tile padding.

Per core:
  - the host ships the stream pre-transposed ([din, pos]); segment-wide
    copies write three operand variants at *shifted column offsets*
    (+1 plain / +2 left-mask / +0 right-mask), so all three conv taps read
    the SAME aligned 128-column slice per output tile and the PE runs
    matmuls only (no transposes).
  - boundary masks fold into the evacuation (tensor_mul vs broadcast mask
    rows); a K=2 matmul adds conv bias and -1e30 on padded tail positions,
    so the single Relu evacuation also zeroes them.
  - matmul operands are float32r (full-rate PE fp32 mode); producers round
    via compute-op copies as the BIR verifier requires.
"""

import math
import sys

for _p in (
    "/opt/trn_rl_repo",
    "/root/.axon_site",
    "/root/.axon_site/_ro/trn_rl_repo",
    "/root/.axon_site/_ro/pypackages",
):
    if _p not in sys.path:
        sys.path.append(_p)

import numpy as np

B, L, D = 32, 1024, 768
C, KCH = 256, 8
NCORES = 8
P = 128
KT = D // P
BIGNEG = -1.0e30
MM_MODE = "f32r"           # "f32r" | "f32"
SP = 4                     # position tiles per SBUF segment

_build_cache = {}


def _build(PTC, mm_mode):
    from contextlib import ExitStack

    import concourse.bacc as bacc
    import concourse.mybir as mybir
    import concourse.tile as tile

    f32 = mybir.dt.float32
    bf16 = mybir.dt.bfloat16
    f32r = mybir.dt.float32r
    fmm = f32r if mm_mode == "f32r" else f32
    NDC = PTC * P
    SEGW = SP * P + 2
    NSEG = (PTC + SP - 1) // SP

    def nblk(s):
        return min(PTC, (s + 1) * SP) - s * SP

    nc = bacc.Bacc("TRN2", target_bir_lowering=False, debug=False)
    xst_d = nc.dram_tensor("xst", [KT, P, NDC], f32, kind="ExternalInput").ap()
    w_d = nc.dram_tensor("w", [3, KT, P, D], f32, kind="ExternalInput").ap()
    mA_d = nc.dram_tensor("maskA", [P, NDC + 2], bf16, kind="ExternalInput").ap()
    mB_d = nc.dram_tensor("maskB", [P, NDC + 2], bf16, kind="ExternalInput").ap()
    blhs_d = nc.dram_tensor("blhs", [2, NDC], f32, kind="ExternalInput").ap()
    brow_d = nc.dram_tensor("brow", [2, D], f32, kind="ExternalInput").ap()
    id_d = nc.dram_tensor("ident", [P, P], f32, kind="ExternalInput").ap()
    out_d = nc.dram_tensor("out", [NDC, D], f32, kind="ExternalOutput").ap()

    with tile.TileContext(nc) as tc:
        with ExitStack() as ctx:
            wp = ctx.enter_context(tc.tile_pool(name="w", bufs=1))
            cst = ctx.enter_context(tc.tile_pool(name="cst", bufs=1))
            xtp = ctx.enter_context(tc.tile_pool(name="xt", bufs=2))
            sgp = ctx.enter_context(tc.tile_pool(name="sg", bufs=4))
            opp = ctx.enter_context(tc.tile_pool(name="op", bufs=4, space="PSUM"))
            outp = ctx.enter_context(tc.tile_pool(name="osb", bufs=5))
            stp = ctx.enter_context(tc.tile_pool(name="stage", bufs=2))

            # --- setup: interleave W-tap DMAs (first-needed first) with the
            # identity, first x tiles, and masks so PE can start early while
            # the 7MB weight wave flows.
            id_sb = cst.tile([P, P], f32, tag="ident", name="ident_sb")
            nc.sync.dma_start(id_sb[:], id_d)
            w_sb = [
                [
                    wp.tile([P, D], fmm, tag=f"w{t}_{k}", name=f"w{t}_{k}")
                    for k in range(KT)
                ]
                for t in range(3)
            ]
            _weng = [nc.vector, nc.scalar, nc.gpsimd]

            def _emit_w(i, t, k):
                if mm_mode == "f32r":
                    ws = stp.tile([P, D], f32, tag="wstage", name="w_stage")
                    nc.sync.dma_start(ws[:], w_d[t, k])
                    eng = _weng[(i * KT + k) % 3]
                    if eng is nc.scalar:
                        eng.copy(w_sb[t][k][:], ws[:])
                    else:
                        eng.tensor_copy(w_sb[t][k][:], ws[:])
                else:
                    nc.sync.dma_start(w_sb[t][k][:], w_d[t, k])

            wu_ps = opp.tile([P, D], f32, tag="po", name="po_ps")
            for j in range(24):
                nc.tensor.matmul(
                    wu_ps[:, 0:P], id_sb[:], id_sb[:], start=True, stop=True
                )
            for i, t in enumerate((1, 0, 2)):
                for k in range(KT):
                    _emit_w(i, t, k)
            mA_sb = cst.tile([P, NDC + 2], bf16, tag="mA", name="mA_sb")
            nc.sync.dma_start(mA_sb[:], mA_d)
            mB_sb = cst.tile([P, NDC + 2], bf16, tag="mB", name="mB_sb")
            nc.sync.dma_start(mB_sb[:], mB_d)
            brow_sb = cst.tile([2, D], fmm, tag="brow", name="brow_sb")
            bl_sb = cst.tile([2, NDC], fmm, tag="bl", name="bl_sb")
            if mm_mode == "f32r":
                brs = cst.tile([2, D], f32, tag="brstage", name="br_stage")
                nc.sync.dma_start(brs[:], brow_d)
                nc.gpsimd.tensor_copy(brow_sb[:], brs[:])
                bls = cst.tile([2, NDC], f32, tag="blstage", name="bl_stage")
                nc.sync.dma_start(bls[:], blhs_d)
                nc.gpsimd.tensor_copy(bl_sb[:], bls[:])
            else:
                nc.sync.dma_start(brow_sb[:], brow_d)
                nc.sync.dma_start(bl_sb[:], blhs_d)
            zt = cst.tile([P, D], f32, tag="zero", name="zero_sb")
            nc.vector.memset(zt[:], 0.0)

            # --- segment XT buffers -------------------------------------
            # col convention per segment (origin block g0): XTc block g at
            # cols [1+128d, +128) (d = g-g0); XTa at [2+128d, ...) holding
            # x[q]*ml[q+1]; XTb at [0+128d, ...) holding x[q]*mr[q-1].
            # All taps for output tile gt read cols [1+128d, +128).
            seg_tiles = [None] * NSEG

            def alloc_seg(s):
                tl = {}
                for v in ("c", "a", "b"):
                    for k in range(KT):
                        tl[v, k] = xtp.tile(
                            [P, SEGW], fmm, tag=f"xt{v}{k}", name=f"xt{v}{k}_s{s}"
                        )
                seg_tiles[s] = tl
                if s == 0:
                    for k in range(KT):
                        nc.gpsimd.tensor_copy(tl["a", k][:, 1:2], zt[:, 0:1])

            def emit_mm_tile(gt):
                s = gt // SP
                d = gt - s * SP
                if True:
                    lc = 1 + d * P
                    po = opp.tile([P, D], f32, tag="po", name="po_ps")
                    for c0, c1 in ((0, 512), (512, D)):
                        n_mm = 3 * KT + 1
                        i = 0
                        for t, v in ((1, "c"), (0, "a"), (2, "b")):
                            for k in range(KT):
                                nc.tensor.matmul(
                                    po[:, c0:c1],
                                    seg_tiles[s][v, k][:, lc : lc + P],
                                    w_sb[t][k][:, c0:c1],
                                    start=(i == 0),
                                    stop=(i == n_mm - 1),
                                )
                                i += 1
                        nc.tensor.matmul(
                            po[:, c0:c1],
                            bl_sb[:, gt * P : (gt + 1) * P],
                            brow_sb[:, c0:c1],
                            start=False,
                            stop=True,
                        )
                    out_sb = outp.tile([P, D], f32, tag="osb", name="out_sb")
                    nc.scalar.activation(
                        out_sb[:], po[:], mybir.ActivationFunctionType.Relu
                    )
                    nc.sync.dma_start(out_d[gt * P : (gt + 1) * P, :], out_sb[:])

            for s in range(NSEG):
                alloc_seg(s)
                g0 = s * SP
                W = nblk(s) * P
                for k in range(KT):
                    xtc = seg_tiles[s]["c", k]
                    st = sgp.tile([P, SP * P], f32, tag="sg", name="sg_st")
                    nc.sync.dma_start(
                        st[:, 0:W], xst_d[k, :, g0 * P : g0 * P + W]
                    )
                    nc.vector.tensor_copy(xtc[:, 1 : 1 + W], st[:, 0:W])
                    nc.vector.tensor_mul(
                        seg_tiles[s]["a", k][:, 2 : 2 + W],
                        xtc[:, 1 : 1 + W],
                        mA_sb[:, g0 * P + 1 : g0 * P + 1 + W],
                    )
                    nc.gpsimd.tensor_mul(
                        seg_tiles[s]["b", k][:, 0:W],
                        xtc[:, 1 : 1 + W],
                        mB_sb[:, g0 * P : g0 * P + W],
                    )
                    if s > 0:
                        nbp = nblk(s - 1) * P
                        # right halo of seg s-1: x[g0*P] * mr[g0*P-1]
                        nc.vector.tensor_mul(
                            seg_tiles[s - 1]["b", k][:, nbp : nbp + 1],
                            xtc[:, 1:2],
                            mB_sb[:, g0 * P : g0 * P + 1],
                        )
                        # left halo of seg s: x[g0*P-1] * ml[g0*P]
                        nc.vector.tensor_mul(
                            seg_tiles[s]["a", k][:, 1:2],
                            seg_tiles[s - 1]["c", k][:, nbp : nbp + 1],
                            mA_sb[:, g0 * P : g0 * P + 1],
                        )
                    if s == NSEG - 1:
                        nc.gpsimd.tensor_copy(
                            seg_tiles[s]["b", k][:, W : W + 1], zt[:, 0:1]
                        )
                if s > 0:
                    for gt in range((s - 1) * SP, s * SP):
                        emit_mm_tile(gt)
            for gt in range((NSEG - 1) * SP, PTC):
                emit_mm_tile(gt)
    nc.compile()
    return nc


def _host_prep(x, cl, conv_w, conv_b):
    """Concat covered tokens globally, split into 8 chunk-aligned spans.

    Returns (PTC, in_maps, scatter) where scatter[core] = (b_idx, p_idx,
    span_len) maps each core's output rows back to (batch, position).
    """
    import ml_dtypes

    x = np.ascontiguousarray(np.asarray(x, np.float32))
    cl = np.asarray(cl).astype(np.int64)
    kept = np.minimum(cl, KCH)
    starts = np.cumsum(cl, axis=1) - cl

    tok_b, tok_p, ml, mr, chunk_start = [], [], [], [], []
    for b in range(B):
        for c in range(C):
            lc = int(kept[b, c])
            if lc <= 0:
                continue
            s = int(starts[b, c])
            chunk_start.append(len(tok_p))
            for j in range(lc):
                tok_b.append(b)
                tok_p.append(s + j)
                ml.append(1.0 if j > 0 else 0.0)
                mr.append(1.0 if j < lc - 1 else 0.0)
    T = len(tok_p)
    tok_b = np.asarray(tok_b, np.int64)
    tok_p = np.asarray(tok_p, np.int64)
    ml = np.asarray(ml, np.float32)
    mr = np.asarray(mr, np.float32)
    cs = np.asarray(chunk_start + [T], np.int64)

    splits = [0]
    for i in range(1, NCORES):
        tgt = round(i * T / NCORES)
        j = int(np.searchsorted(cs, tgt, side="right")) - 1
        splits.append(int(cs[max(j, 0)]))
    splits.append(T)
    span_len = [splits[i + 1] - splits[i] for i in range(NCORES)]
    PTC = max(1, int(math.ceil(max(span_len) / P)))
    NDC = PTC * P

    w_prep = np.ascontiguousarray(
        np.transpose(np.asarray(conv_w, np.float32), (2, 1, 0)).reshape(3, KT, P, D)
    )
    brow = np.empty((2, D), np.float32)
    brow[0] = np.asarray(conv_b, np.float32)
    brow[1] = BIGNEG
    ident = np.eye(P, dtype=np.float32)

    in_maps, scatter = [], []
    for core in range(NCORES):
        lo, hi = splits[core], splits[core + 1]
        n = hi - lo
        xs = np.zeros((NDC, D), np.float32)
        xs[:n] = x[tok_b[lo:hi], tok_p[lo:hi]]
        xst = np.ascontiguousarray(xs.T).reshape(KT, P, NDC)
        rowA = np.zeros(NDC + 2, np.float32)
        rowA[:n] = ml[lo:hi]                      # maskA[i] = ml[i]
        rowB = np.zeros(NDC + 2, np.float32)
        rowB[1 : 1 + n] = mr[lo:hi]               # maskB[i] = mr[i-1]
        blhs = np.zeros((2, NDC), np.float32)
        blhs[0] = 1.0
        blhs[1, n:] = 1.0                         # invalid tail -> -1e30
        in_maps.append(
            {
                "xst": xst,
                "w": w_prep,
                "maskA": np.ascontiguousarray(
                    np.broadcast_to(rowA, (P, NDC + 2))
                ).astype(ml_dtypes.bfloat16),
                "maskB": np.ascontiguousarray(
                    np.broadcast_to(rowB, (P, NDC + 2))
                ).astype(ml_dtypes.bfloat16),
                "blhs": blhs,
                "brow": brow,
                "ident": ident,
            }
        )
        scatter.append((tok_b[lo:hi], tok_p[lo:hi], n))
    return PTC, in_maps, scatter


def _get_nc(PTC, mm_mode=None):
    mm_mode = mm_mode or MM_MODE
    key = (PTC, mm_mode, SP)
    if key not in _build_cache:
        _build_cache[key] = _build(PTC, mm_mode)
    return _build_cache[key]


def kernel(**inputs):
    from concourse.bass_utils import run_bass_kernel_spmd

    PTC, in_maps, scatter = _host_prep(
        inputs["token_level_features"],
        inputs["chunk_lens"],
        inputs["conv_w"],
        inputs["conv_b"],
    )
    nc = _get_nc(PTC)
    res = run_bass_kernel_spmd(nc, in_maps, core_ids=list(range(NCORES)))
    out = np.zeros((B, L, D), np.float32)
    for core in range(NCORES):
        bi, pi, n = scatter[core]
        out[bi, pi] = res.results[core]["out"][:n]
    return out
